# revision 27
# baseline (speedup 1.0000x reference)
"""GQA attention (B=1, T=2048, D=2048, H=32, KVH=8, HD=64) on 8 TRN2 cores.

Head-tensor-parallel: core c owns kv-head c and q-heads 4c..4c+3.
wq/wk/wv column-parallel, wo row-parallel; partials summed on host.

Pipeline: A (kv proj + rope-k + v transpose) and B (q proj + rope-q +
perm) interleaved per 512-column chunk with KV/E/O accumulation
round-robined across PSUM banks; C (attention) with two head-pair
streams so ACT exp overlaps PE, causal diagonal blocks column-restricted;
vx carries 64 ones-columns so the pv matmul broadcasts the softmax
denominator for free; D (wo proj) with dd-interleaved PSUM banks.
"""
import sys

if "/opt/trn_rl_repo" not in sys.path:
    sys.path.insert(0, "/opt/trn_rl_repo")

import numpy as np
import ml_dtypes

import concourse.bacc as bacc
import concourse.mybir as mybir
import concourse.tile as tile
from concourse.bass_utils import run_bass_kernel_spmd

BF16 = ml_dtypes.bfloat16
T, D, H, KVH, HD = 2048, 2048, 32, 8, 64
NCORES = 8
HPC = H // NCORES            # 4 q heads per core
KT, PT = 16, 128             # k-tiles of 128 over D
NCH = 4                      # t chunks of 512
CH = 512

_cache = {}


def _build_nc():
    if "nc" in _cache:
        return _cache["nc"]
    fp32, bf16 = mybir.dt.float32, mybir.dt.bfloat16
    Exp = mybir.ActivationFunctionType.Exp
    mult = mybir.AluOpType.mult
    nc = bacc.Bacc("TRN2", target_bir_lowering=False, debug=False,
                   num_devices=NCORES)

    xt_d = nc.dram_tensor("xt", [D, T], bf16, kind="ExternalInput")
    wq_d = nc.dram_tensor("wq", [PT, KT * HPC * HD], bf16, kind="ExternalInput")
    wkv_d = nc.dram_tensor("wkv", [PT, KT * 2 * HD], bf16, kind="ExternalInput")
    wo_d = nc.dram_tensor("wo", [PT, 2 * D], bf16, kind="ExternalInput")
    cs4_d = nc.dram_tensor("cs4", [PT, T], bf16, kind="ExternalInput")
    sn4_d = nc.dram_tensor("sn4", [PT, T], bf16, kind="ExternalInput")
    id_d = nc.dram_tensor("ident", [64, 64], bf16, kind="ExternalInput")
    lm_d = nc.dram_tensor("lmask", [PT, PT], bf16, kind="ExternalInput")
    i128_d = nc.dram_tensor("id128", [PT, PT], bf16, kind="ExternalInput")
    out_d = nc.dram_tensor("partial", [T, D], bf16, kind="ExternalOutput")

    with tile.TileContext(nc) as tc:
        with tc.tile_pool(name="const", bufs=1) as const, \
             tc.tile_pool(name="xtp", bufs=4 * KT) as xtp, \
             tc.tile_pool(name="persist", bufs=1) as persist:

            # ---- DMA loads; sync queue: weights/consts + chunk-0 x,
            # gpsimd queue: gpsimd lib warmup then x chunks 1-3 (queues
            # generate descriptors serially, so split across engines) ----
            wkv_sb = const.tile([PT, KT, 2 * HD], bf16, tag="wkv")
            nc.sync.dma_start(wkv_sb[:], wkv_d.ap())
            wq_sb = const.tile([PT, KT, HPC * HD], bf16, tag="wq")
            # x tiles split across the sync and gpsimd queues (each queue
            # issues descriptors serially at ~0.7us) in chunk-priority
            # order so chunk j lands before chunk j+1 starts competing.
            xt = [[None] * KT for _ in range(NCH)]
            consts_emitted = [False]

            def _emit_consts():
                consts_emitted[0] = True
                out = {}
                out["cs4"] = const.tile([PT, T], bf16, tag="cs4", name="cs4")
                nc.sync.dma_start(out["cs4"][:], cs4_d.ap())
                out["sn4"] = const.tile([PT, T], bf16, tag="sn4", name="sn4")
                nc.sync.dma_start(out["sn4"][:], sn4_d.ap())
                out["ident"] = const.tile([64, 64], bf16, tag="ident", name="ident")
                nc.sync.dma_start(out["ident"][:], id_d.ap())
                out["lmask"] = const.tile([PT, PT], bf16, tag="lmask", name="lmask")
                nc.sync.dma_start(out["lmask"][:], lm_d.ap())
                out["id128"] = const.tile([PT, PT], bf16, tag="id128", name="id128")
                nc.sync.dma_start(out["id128"][:], i128_d.ap())
                return out

            for j in range(NCH):
                for k in range(KT):
                    t_ = xtp.tile([PT, CH], bf16, tag="xt", name=f"xt_{j}_{k}")
                    eng = nc.sync if k % 2 == 0 else nc.gpsimd
                    eng.dma_start(
                        t_[:], xt_d.ap()[k * PT:(k + 1) * PT, j * CH:(j + 1) * CH])
                    xt[j][k] = t_
                    if j == 0 and k == 7:
                        # wq lands after the first chunk-0 x tiles: the E
                        # matmuls only need it once KV(0) is underway
                        nc.sync.dma_start(wq_sb[:], wq_d.ap())
                if j == 0:
                    cm = _emit_consts()
            cs4, sn4, ident = cm["cs4"], cm["sn4"], cm["ident"]
            lmask, id128 = cm["lmask"], cm["id128"]
            wo_sb = const.tile([PT, 2, D], bf16, tag="wo")
            nc.sync.dma_start(wo_sb[:], wo_d.ap())

            # ---- persistent SBUF activations ----
            kt = persist.tile([64, T], bf16, tag="kt")
            # vx: [seq, v-dims 0:64 | ones 64:128]; the ones columns make the
            # pv matmul emit the softmax denominator broadcast to rows 64:128.
            vx = persist.tile([PT, KT, PT], bf16, tag="vx")
            nc.vector.memset(vx[:, :, HD:PT], 1.0)
            qtc = [persist.tile([64, HPC * CH], bf16, tag=f"qtc{j}", name=f"qtc{j}")
                   for j in range(NCH)]
            ot = [persist.tile([PT, T], bf16, tag=f"ot{s}", name=f"ot{s}")
                  for s in range(2)]

            # ---- phase A || B: projections + rope, chunk by chunk ----
            # PSUM banks: kv bufs=2 (2) + E,O bufs=2 (4) + vtr (1) = 7
            with tc.tile_pool(name="kvp", bufs=2, space="PSUM") as kvp, \
                 tc.tile_pool(name="eop", bufs=2, space="PSUM") as eop, \
                 tc.tile_pool(name="vtp", bufs=1, space="PSUM") as vtp, \
                 tc.tile_pool(name="tmpab", bufs=2) as tmpab:
                vts = {}

                def emit_tr(jj):
                    # v transpose for chunk jj (vt ready by now)
                    vt = vts.pop(jj)
                    vtr = vtp.tile([PT, 4, HD], bf16, tag="vtr")
                    for u in range(4):
                        nc.tensor.transpose(vtr[:, u, :], vt[:, u * PT:(u + 1) * PT],
                                            ident[:])
                    nc.vector.tensor_copy(vx[:, 4 * jj:4 * jj + 4, 0:HD], vtr[:])

                for j in range(NCH):
                    jsl = slice(j * CH, (j + 1) * CH)
                    KV = kvp.tile([PT, CH], fp32, tag="kv", name=f"kv{j}")
                    E = eop.tile([PT, CH], fp32, tag="E", name=f"E{j}")
                    O = eop.tile([PT, CH], fp32, tag="O", name=f"O{j}")
                    if j >= 1:
                        emit_tr(j - 1)
                    # KV/E/O k-loop round-robined across 3 banks (hides the
                    # psum read-modify-write latency of accumulation). Chunk 0
                    # is DMA-paced: run KV alone first so the E matmuls never
                    # block the in-order PE queue waiting for wq to land.
                    if j == 0:
                        for k in range(KT):
                            nc.tensor.matmul(KV[:], wkv_sb[:, k, :], xt[j][k][:],
                                             start=(k == 0), stop=(k == KT - 1))
                        for k in range(KT):
                            st, sp = (k == 0), (k == KT - 1)
                            nc.tensor.matmul(E[:], wq_sb[:, k, 0:PT], xt[j][k][:],
                                             start=st, stop=sp)
                            nc.tensor.matmul(O[:], wq_sb[:, k, PT:2 * PT],
                                             xt[j][k][:], start=st, stop=sp)
                    else:
                        for k in range(KT):
                            st, sp = (k == 0), (k == KT - 1)
                            nc.tensor.matmul(KV[:], wkv_sb[:, k, :], xt[j][k][:],
                                             start=st, stop=sp)
                            nc.tensor.matmul(E[:], wq_sb[:, k, 0:PT], xt[j][k][:],
                                             start=st, stop=sp)
                            nc.tensor.matmul(O[:], wq_sb[:, k, PT:2 * PT],
                                             xt[j][k][:], start=st, stop=sp)
                    # v extract: vt copy on ACT
                    vt = tmpab.tile([64, CH], bf16, tag="vt")
                    nc.scalar.copy(vt[:], KV[64:PT, :])
                    vts[j] = vt
                    # rope-q (DVE) first -> rE/rO bf16 (perm of the NEXT
                    # chunk blocks on these; rope-k is only needed by C)
                    t1 = tmpab.tile([PT, CH], fp32, tag="t1")
                    t3 = tmpab.tile([PT, CH], fp32, tag="t3")
                    nc.vector.tensor_tensor(t1[:], E[:], cs4[:, jsl], mult)
                    nc.vector.tensor_tensor(t3[:], E[:], sn4[:, jsl], mult)
                    t2 = tmpab.tile([PT, CH], fp32, tag="t2")
                    t4 = tmpab.tile([PT, CH], fp32, tag="t4")
                    nc.vector.tensor_tensor(t2[:], O[:], sn4[:, jsl], mult)
                    nc.vector.tensor_tensor(t4[:], O[:], cs4[:, jsl], mult)
                    # write qtc directly on gpsimd (inputs are SBUF fp32;
                    # keeps the DVE free for rope-k and the C-phase norm):
                    # head h rows 0:32 = t1-t2 rows 32h.., 32:64 = t3+t4
                    for h in range(HPC):
                        hp = slice(32 * h, 32 * h + 32)
                        nc.gpsimd.tensor_tensor(
                            qtc[j][0:32, h * CH:(h + 1) * CH],
                            t1[hp, :], t2[hp, :], mybir.AluOpType.subtract)
                        nc.gpsimd.tensor_tensor(
                            qtc[j][32:64, h * CH:(h + 1) * CH],
                            t3[hp, :], t4[hp, :], mybir.AluOpType.add)
                    # rope-k (DVE): kt rows 0:32 = Kev*c - Kod*s ; 32:64 = Kev*s + Kod*c
                    k1 = tmpab.tile([32, CH], fp32, tag="k1")
                    k2 = tmpab.tile([32, CH], fp32, tag="k2")
                    nc.vector.tensor_tensor(k1[:], KV[0:32, :], cs4[0:32, jsl], mult)
                    nc.vector.tensor_tensor(k2[:], KV[32:64, :], sn4[0:32, jsl], mult)
                    nc.vector.tensor_sub(kt[0:32, jsl], k1[:], k2[:])
                    k3 = tmpab.tile([32, CH], fp32, tag="k1")
                    k4 = tmpab.tile([32, CH], fp32, tag="k2")
                    nc.vector.tensor_tensor(k3[:], KV[0:32, :], sn4[0:32, jsl], mult)
                    nc.vector.tensor_tensor(k4[:], KV[32:64, :], cs4[0:32, jsl], mult)
                    nc.vector.tensor_add(kt[32:64, jsl], k3[:], k4[:])
                emit_tr(NCH - 1)

            # ---- phase C: attention, software-pipelined ----
            # PSUM: scA/scB [128,2,512] = 2+2 banks, pv [128,4,512] = 4 banks.
            # Two head-pair streams so exp(pairA) overlaps scores(pairB); pv
            # matmuls lag one i behind scores. Per chunk j: diagonal blocks
            # first (col-restricted to the causal-live columns).
            with tc.tile_pool(name="scp", bufs=2, space="PSUM") as scp, \
                 tc.tile_pool(name="pvp", bufs=1, space="PSUM") as pvp, \
                 tc.tile_pool(name="exq", bufs=4) as exq, \
                 tc.tile_pool(name="nrm", bufs=2) as nrm:
                scg = [scp.tile([PT, 2, CH], fp32, tag="sc", name=f"sc{g}")
                       for g in range(2)]
                pv = pvp.tile([PT, HPC, CH], fp32, tag="pv")
                pend = []  # (i, c0, [exA, exB], start, stop, j_done)

                def emit_norm(jj):
                    # normalization per head: denominator rows -> SBUF ->
                    # recip (reciprocal_approx_fast requires SBUF input),
                    # values scaled straight from psum.
                    for h in range(HPC):
                        dn = nrm.tile([64, CH], fp32, tag="dn")
                        nc.vector.tensor_copy(dn[:], pv[64:PT, h, :])
                        rcp = nrm.tile([64, CH], fp32, tag="rcp")
                        nc.vector.reciprocal_approx_fast(rcp[:], dn[:])
                        nc.vector.tensor_tensor(
                            ot[h // 2][64 * (h % 2):64 * (h % 2) + 64,
                                       jj * CH:(jj + 1) * CH],
                            pv[0:HD, h, :], rcp[:], mult)

                half = [None]

                def flush_pv():
                    # chunk-start items flush in two halves (h 0:2, then
                    # h 2:4) so the fresh norm gets time to release banks
                    if half[0] is not None:
                        i, c0, exg, st, sp, jdone = half[0]
                        hs = range(2, HPC)
                        half[0] = None
                    else:
                        item = pend.pop(0)
                        i, c0, exg, st, sp, jdone = item
                        if st:
                            half[0] = item
                            hs = range(0, 2)
                        else:
                            hs = range(HPC)
                    for h in hs:
                        nc.tensor.matmul(
                            pv[:, h, c0:CH], vx[:, i, :],
                            exg[h // 2][:, h % 2, c0:CH],
                            start=st, stop=sp)
                    if jdone is not None and half[0] is None:
                        emit_norm(jdone)

                for j in range(NCH):
                    idxs = list(range(4 * j, 4 * j + 4)) + list(range(4 * j))
                    nlast = len(idxs) - 1
                    for idx, i in enumerate(idxs):
                        r = i - 4 * j if i >= 4 * j else None
                        c0 = PT * r if r is not None else 0
                        ktsl = kt[:, i * PT:(i + 1) * PT]
                        exg = []
                        for g in range(2):
                            diag = r is not None
                            for hh in range(2):
                                h = 2 * g + hh
                                nc.tensor.matmul(
                                    scg[g][:, hh, c0:CH], ktsl,
                                    qtc[j][:, h * CH + c0:(h + 1) * CH],
                                    start=True, stop=not diag)
                            if diag:
                                # additive causal mask: sc[s, c'] += -1e4
                                # for c' < s on the triangle block, so exp
                                # yields exact zeros (nothing extra in the
                                # exp -> pv chain)
                                for hh in range(2):
                                    nc.tensor.matmul(
                                        scg[g][:, hh, c0:c0 + PT], lmask[:],
                                        id128[:], start=False, stop=True)
                            ex = exq.tile([PT, 2, CH], bf16, tag=f"ex{g}",
                                          name=f"ex{g}")
                            nc.scalar.activation(ex[:, :, c0:CH],
                                                 scg[g][:, :, c0:CH],
                                                 Exp, scale=0.125)
                            exg.append(ex)
                        pend.append((i, c0, exg, idx == 0, idx == nlast,
                                     j if idx == nlast else None))
                        # constant lag of 3: a chunk's last pv flushes (and
                        # its norm) interleave with the NEXT chunk's first
                        # score matmuls, so ACT never starves at boundaries.
                        while len(pend) > 3:
                            flush_pv()
                while pend or half[0] is not None:
                    flush_pv()

            # ---- phase D: output projection, dd-interleaved banks ----
            with tc.tile_pool(name="wp", bufs=4, space="PSUM") as wpp, \
                 tc.tile_pool(name="po", bufs=3) as pop:
                n = 0
                for tt in range(KT):
                    wps = [wpp.tile([PT, CH], fp32, tag="wp", name=f"wp{tt}_{dd}")
                           for dd in range(NCH)]
                    for s in range(2):
                        for dd in range(NCH):
                            nc.tensor.matmul(wps[dd][:],
                                             ot[s][:, tt * PT:(tt + 1) * PT],
                                             wo_sb[:, s, dd * CH:(dd + 1) * CH],
                                             start=(s == 0), stop=(s == 1))
                    pout4 = pop.tile([PT, NCH, CH], bf16, tag="po")
                    for dd in range(NCH):
                        if n % 2 == 0:
                            nc.scalar.copy(pout4[:, dd, :], wps[dd][:])
                        else:
                            nc.vector.tensor_copy(pout4[:, dd, :], wps[dd][:])
                        n += 1
                    nc.sync.dma_start(
                        out_d.ap()[tt * PT:(tt + 1) * PT, :], pout4[:])

    nc.compile()
    _cache["nc"] = nc
    return nc


def _host_prep(x, freqs, wq, wk, wv, wo):
    x2d = np.asarray(x, np.float32)[0]                    # [T, D]
    xt = np.ascontiguousarray(x2d.T).astype(BF16)         # [D, T]
    cos = np.cos(np.asarray(freqs, np.float32))           # [T, 32]
    sin = np.sin(np.asarray(freqs, np.float32))
    cs4 = np.ascontiguousarray(np.tile(cos.T, (4, 1)))    # [128, T]
    sn4 = np.ascontiguousarray(np.tile(sin.T, (4, 1)))

    ev, od = np.arange(0, HD, 2), np.arange(1, HD, 2)

    # permE/permO [128, 256]: head h (cols 64h..64h+63): local row r<32 comes
    # from rE row 32h+r, r>=32 from rO row 32h+(r-32)
    permE = np.zeros((PT, 2 * PT), np.float32)
    permO = np.zeros((PT, 2 * PT), np.float32)
    for h in range(HPC):
        for r in range(32):
            permE[32 * h + r, 64 * h + r] = 1.0
            permO[32 * h + r, 64 * h + 32 + r] = 1.0

    ident = np.eye(64, dtype=np.float32)

    # lmask [128, 128]: lmask[k, s] = -1e4 where k < s; via identity rhs
    # the matmul adds -1e4 to score[s, c'] for c' < s (causal mask)
    kk = np.arange(PT)[:, None]
    ss = np.arange(PT)[None, :]
    lmask = np.where(kk < ss, -1.0e4, 0.0).astype(np.float32)
    id128 = np.eye(PT, dtype=np.float32)

    wq_f = np.asarray(wq, np.float32)
    wk_f = np.asarray(wk, np.float32)
    wv_f = np.asarray(wv, np.float32)
    wo_f = np.asarray(wo, np.float32)

    in_maps = []
    for c in range(NCORES):
        # wq for 4 heads, evens-major-across-heads packing:
        # cols 0:128 = [h0 evens, h1 evens, h2 evens, h3 evens], 128:256 odds
        blocks = [wq_f[:, (c * HPC + h) * HD:(c * HPC + h + 1) * HD] for h in range(HPC)]
        wq_c = np.concatenate([b[:, ev] for b in blocks] + [b[:, od] for b in blocks], axis=1)
        kblk = wk_f[:, c * HD:(c + 1) * HD]
        wkv_c = np.concatenate([kblk[:, ev], kblk[:, od],
                                wv_f[:, c * HD:(c + 1) * HD]], axis=1)
        wo_c = wo_f[c * HPC * HD:(c + 1) * HPC * HD, :]
        # pre-tile to [128 partitions, k-major] so device DMAs are contiguous
        wq_c = wq_c.reshape(KT, PT, HPC * HD).transpose(1, 0, 2).reshape(PT, -1)
        wkv_c = wkv_c.reshape(KT, PT, 2 * HD).transpose(1, 0, 2).reshape(PT, -1)
        wo_c = wo_c.reshape(2, PT, D).transpose(1, 0, 2).reshape(PT, -1)
        in_maps.append({
            "xt": xt,
            "wq": np.ascontiguousarray(wq_c).astype(BF16),
            "wkv": np.ascontiguousarray(wkv_c).astype(BF16),
            "wo": np.ascontiguousarray(wo_c).astype(BF16),
            "cs4": cs4.astype(BF16),
            "sn4": sn4.astype(BF16),
            "ident": ident.astype(BF16),
            "lmask": lmask.astype(BF16),
            "id128": id128.astype(BF16),
        })
    return in_maps


def run(inputs, trace=False, tmpdir=None):
    nc = _build_nc()
    in_maps = _host_prep(**inputs)
    res = run_bass_kernel_spmd(nc, in_maps, list(range(NCORES)),
                               trace=trace, tmpdir=tmpdir)
    acc = np.zeros((T, D), np.float32)
    for c in range(NCORES):
        acc += res.results[c]["partial"].astype(np.float32)
    return acc[None], res


def kernel(**inputs):
    out, _ = run(inputs, trace=False)
    return out


# revision 28
# speedup vs baseline: 1.0844x; 1.0844x over previous
"""GQA attention (B=1, T=2048, D=2048, H=32, KVH=8, HD=64) on 8 TRN2 cores.

Head-tensor-parallel: core c owns kv-head c and q-heads 4c..4c+3.
wq/wk/wv column-parallel, wo row-parallel; partials summed on host.

Pipeline: A (kv proj + rope-k + v transpose) and B (q proj + rope-q +
perm) interleaved per 512-column chunk with KV/E/O accumulation
round-robined across PSUM banks; C (attention) with two head-pair
streams so ACT exp overlaps PE, causal diagonal blocks column-restricted;
vx carries 64 ones-columns so the pv matmul broadcasts the softmax
denominator for free; D (wo proj) with dd-interleaved PSUM banks.
"""
import sys

if "/opt/trn_rl_repo" not in sys.path:
    sys.path.insert(0, "/opt/trn_rl_repo")

import numpy as np
import ml_dtypes

import concourse.bacc as bacc
import concourse.mybir as mybir
import concourse.tile as tile
from concourse.bass_utils import run_bass_kernel_spmd

BF16 = ml_dtypes.bfloat16
T, D, H, KVH, HD = 2048, 2048, 32, 8, 64
NCORES = 8
HPC = H // NCORES            # 4 q heads per core
KT, PT = 16, 128             # k-tiles of 128 over D
NCH = 4                      # t chunks of 512
CH = 512

_cache = {}


def _build_nc():
    if "nc" in _cache:
        return _cache["nc"]
    fp32, bf16 = mybir.dt.float32, mybir.dt.bfloat16
    Exp = mybir.ActivationFunctionType.Exp
    mult = mybir.AluOpType.mult
    nc = bacc.Bacc("TRN2", target_bir_lowering=False, debug=False,
                   num_devices=NCORES)

    xt_d = nc.dram_tensor("xt", [D, T], bf16, kind="ExternalInput")
    wq_d = nc.dram_tensor("wq", [PT, KT * HPC * HD], bf16, kind="ExternalInput")
    wkv_d = nc.dram_tensor("wkv", [PT, KT * 2 * HD], bf16, kind="ExternalInput")
    wo_d = nc.dram_tensor("wo", [PT, 2 * D], bf16, kind="ExternalInput")
    cs4_d = nc.dram_tensor("cs4", [PT, T], bf16, kind="ExternalInput")
    sn4_d = nc.dram_tensor("sn4", [PT, T], bf16, kind="ExternalInput")
    id_d = nc.dram_tensor("ident", [64, 64], bf16, kind="ExternalInput")
    lm_d = nc.dram_tensor("lmask", [PT, PT], bf16, kind="ExternalInput")
    i128_d = nc.dram_tensor("id128", [PT, PT], bf16, kind="ExternalInput")
    out_d = nc.dram_tensor("partial", [T, D], bf16, kind="ExternalOutput")

    with tile.TileContext(nc) as tc:
        with tc.tile_pool(name="const", bufs=1) as const, \
             tc.tile_pool(name="xtp", bufs=4 * KT) as xtp, \
             tc.tile_pool(name="persist", bufs=1) as persist:

            # ---- DMA loads; sync queue: weights/consts + chunk-0 x,
            # gpsimd queue: gpsimd lib warmup then x chunks 1-3 (queues
            # generate descriptors serially, so split across engines) ----
            wkv_sb = const.tile([PT, KT, 2 * HD], bf16, tag="wkv")
            nc.sync.dma_start(wkv_sb[:], wkv_d.ap())
            wq_sb = const.tile([PT, KT, HPC * HD], bf16, tag="wq")
            # x tiles split across the sync and gpsimd queues (each queue
            # issues descriptors serially at ~0.7us) in chunk-priority
            # order so chunk j lands before chunk j+1 starts competing.
            xt = [[None] * KT for _ in range(NCH)]
            consts_emitted = [False]

            def _emit_consts():
                consts_emitted[0] = True
                out = {}
                out["cs4"] = const.tile([PT, T], bf16, tag="cs4", name="cs4")
                nc.sync.dma_start(out["cs4"][:], cs4_d.ap())
                out["sn4"] = const.tile([PT, T], bf16, tag="sn4", name="sn4")
                nc.sync.dma_start(out["sn4"][:], sn4_d.ap())
                out["ident"] = const.tile([64, 64], bf16, tag="ident", name="ident")
                nc.sync.dma_start(out["ident"][:], id_d.ap())
                out["lmask"] = const.tile([PT, PT], bf16, tag="lmask", name="lmask")
                nc.sync.dma_start(out["lmask"][:], lm_d.ap())
                out["id128"] = const.tile([PT, PT], bf16, tag="id128", name="id128")
                nc.sync.dma_start(out["id128"][:], i128_d.ap())
                return out

            for j in range(NCH):
                for k in range(KT):
                    t_ = xtp.tile([PT, CH], bf16, tag="xt", name=f"xt_{j}_{k}")
                    eng = nc.sync if k % 2 == 0 else nc.gpsimd
                    eng.dma_start(
                        t_[:], xt_d.ap()[k * PT:(k + 1) * PT, j * CH:(j + 1) * CH])
                    xt[j][k] = t_
                    if j == 0 and k == 7:
                        # wq lands after the first chunk-0 x tiles: the E
                        # matmuls only need it once KV(0) is underway
                        nc.sync.dma_start(wq_sb[:], wq_d.ap())
                if j == 0:
                    cm = _emit_consts()
            cs4, sn4, ident = cm["cs4"], cm["sn4"], cm["ident"]
            lmask, id128 = cm["lmask"], cm["id128"]
            wo_sb = const.tile([PT, 2, D], bf16, tag="wo")
            nc.sync.dma_start(wo_sb[:], wo_d.ap())

            # ---- persistent SBUF activations ----
            kt = persist.tile([64, T], bf16, tag="kt")
            # vx: [seq, v-dims 0:64 | ones 64:128]; the ones columns make the
            # pv matmul emit the softmax denominator broadcast to rows 64:128.
            vx = persist.tile([PT, KT, PT], bf16, tag="vx")
            nc.vector.memset(vx[:, :, HD:PT], 1.0)
            qtc = [persist.tile([64, HPC * CH], bf16, tag=f"qtc{j}", name=f"qtc{j}")
                   for j in range(NCH)]
            ot = [persist.tile([PT, T], bf16, tag=f"ot{s}", name=f"ot{s}")
                  for s in range(2)]

            # ---- phase A || B: projections + rope, chunk by chunk ----
            # PSUM banks: kv bufs=2 (2) + E,O bufs=2 (4) + vtr (1) = 7
            with tc.tile_pool(name="kvp", bufs=2, space="PSUM") as kvp, \
                 tc.tile_pool(name="eop", bufs=2, space="PSUM") as eop, \
                 tc.tile_pool(name="vtp", bufs=1, space="PSUM") as vtp, \
                 tc.tile_pool(name="tmpab", bufs=2) as tmpab:
                vts = {}

                def emit_tr(jj):
                    # v transpose for chunk jj (vt ready by now)
                    vt = vts.pop(jj)
                    vtr = vtp.tile([PT, 4, HD], bf16, tag="vtr")
                    for u in range(4):
                        nc.tensor.transpose(vtr[:, u, :], vt[:, u * PT:(u + 1) * PT],
                                            ident[:])
                    nc.scalar.copy(vx[:, 4 * jj:4 * jj + 4, 0:HD], vtr[:])

                for j in range(NCH):
                    jsl = slice(j * CH, (j + 1) * CH)
                    KV = kvp.tile([PT, CH], fp32, tag="kv", name=f"kv{j}")
                    E = eop.tile([PT, CH], fp32, tag="E", name=f"E{j}")
                    O = eop.tile([PT, CH], fp32, tag="O", name=f"O{j}")
                    if j >= 1:
                        emit_tr(j - 1)
                    # KV/E/O k-loop round-robined across 3 banks (hides the
                    # psum read-modify-write latency of accumulation). Chunk 0
                    # is DMA-paced: run KV alone first so the E matmuls never
                    # block the in-order PE queue waiting for wq to land.
                    if j == 0:
                        for k in range(KT):
                            nc.tensor.matmul(KV[:], wkv_sb[:, k, :], xt[j][k][:],
                                             start=(k == 0), stop=(k == KT - 1))
                        for k in range(KT):
                            st, sp = (k == 0), (k == KT - 1)
                            nc.tensor.matmul(E[:], wq_sb[:, k, 0:PT], xt[j][k][:],
                                             start=st, stop=sp)
                            nc.tensor.matmul(O[:], wq_sb[:, k, PT:2 * PT],
                                             xt[j][k][:], start=st, stop=sp)
                    else:
                        for k in range(KT):
                            st, sp = (k == 0), (k == KT - 1)
                            nc.tensor.matmul(KV[:], wkv_sb[:, k, :], xt[j][k][:],
                                             start=st, stop=sp)
                            nc.tensor.matmul(E[:], wq_sb[:, k, 0:PT], xt[j][k][:],
                                             start=st, stop=sp)
                            nc.tensor.matmul(O[:], wq_sb[:, k, PT:2 * PT],
                                             xt[j][k][:], start=st, stop=sp)
                    # v extract: vt copy on ACT
                    vt = tmpab.tile([64, CH], bf16, tag="vt")
                    nc.scalar.copy(vt[:], KV[64:PT, :])
                    vts[j] = vt
                    # rope-q (DVE) first -> rE/rO bf16 (perm of the NEXT
                    # chunk blocks on these; rope-k is only needed by C)
                    t1 = tmpab.tile([PT, CH], fp32, tag="t1")
                    t3 = tmpab.tile([PT, CH], fp32, tag="t3")
                    nc.vector.tensor_tensor(t1[:], E[:], cs4[:, jsl], mult)
                    nc.vector.tensor_tensor(t3[:], E[:], sn4[:, jsl], mult)
                    t2 = tmpab.tile([PT, CH], fp32, tag="t2")
                    t4 = tmpab.tile([PT, CH], fp32, tag="t4")
                    nc.vector.tensor_tensor(t2[:], O[:], sn4[:, jsl], mult)
                    nc.vector.tensor_tensor(t4[:], O[:], cs4[:, jsl], mult)
                    # rope-k mults next: last readers of the KV psum bank
                    # (release it early); the kt / qtc writes are pure-SBUF
                    # and only needed by phase C, so they trail
                    k1 = tmpab.tile([32, CH], fp32, tag="k1")
                    k2 = tmpab.tile([32, CH], fp32, tag="k2")
                    k3 = tmpab.tile([32, CH], fp32, tag="k3")
                    k4 = tmpab.tile([32, CH], fp32, tag="k4")
                    nc.vector.tensor_tensor(k1[:], KV[0:32, :], cs4[0:32, jsl], mult)
                    nc.vector.tensor_tensor(k2[:], KV[32:64, :], sn4[0:32, jsl], mult)
                    nc.vector.tensor_tensor(k3[:], KV[0:32, :], sn4[0:32, jsl], mult)
                    nc.vector.tensor_tensor(k4[:], KV[32:64, :], cs4[0:32, jsl], mult)
                    nc.vector.tensor_sub(kt[0:32, jsl], k1[:], k2[:])
                    nc.vector.tensor_add(kt[32:64, jsl], k3[:], k4[:])
                    # write qtc directly: head h rows 0:32 = t1-t2 rows
                    # 32h.., rows 32:64 = t3+t4 (inputs share a start
                    # partition; output start may differ)
                    for h in range(HPC):
                        hp = slice(32 * h, 32 * h + 32)
                        nc.vector.tensor_sub(qtc[j][0:32, h * CH:(h + 1) * CH],
                                             t1[hp, :], t2[hp, :])
                        nc.vector.tensor_add(qtc[j][32:64, h * CH:(h + 1) * CH],
                                             t3[hp, :], t4[hp, :])
                emit_tr(NCH - 1)

            # ---- phase C: attention, software-pipelined ----
            # PSUM: scA/scB [128,2,512] = 2+2 banks, pv [128,4,512] = 4 banks.
            # Two head-pair streams so exp(pairA) overlaps scores(pairB); pv
            # matmuls lag one i behind scores. Per chunk j: diagonal blocks
            # first (col-restricted to the causal-live columns).
            with tc.tile_pool(name="scp", bufs=2, space="PSUM") as scp, \
                 tc.tile_pool(name="pvp", bufs=1, space="PSUM") as pvp, \
                 tc.tile_pool(name="exq", bufs=4) as exq, \
                 tc.tile_pool(name="nrm", bufs=2) as nrm:
                scg = [scp.tile([PT, 2, CH], fp32, tag="sc", name=f"sc{g}")
                       for g in range(2)]
                pv = pvp.tile([PT, HPC, CH], fp32, tag="pv")
                pend = []  # (i, c0, [exA, exB], start, stop, j_done)

                def emit_norm(jj):
                    # normalization per head: denominator rows -> SBUF ->
                    # recip (reciprocal_approx_fast requires SBUF input),
                    # values scaled straight from psum.
                    for h in range(HPC):
                        dn = nrm.tile([64, CH], fp32, tag="dn")
                        nc.vector.tensor_copy(dn[:], pv[64:PT, h, :])
                        rcp = nrm.tile([64, CH], fp32, tag="rcp")
                        nc.vector.reciprocal_approx_fast(rcp[:], dn[:])
                        nc.vector.tensor_tensor(
                            ot[h // 2][64 * (h % 2):64 * (h % 2) + 64,
                                       jj * CH:(jj + 1) * CH],
                            pv[0:HD, h, :], rcp[:], mult)

                part = [None, 0]

                def flush_pv():
                    # chunk-start items flush one head per call so the
                    # fresh norm gets time to release each pv bank
                    if part[0] is not None:
                        i, c0, exg, st, sp, jdone = part[0]
                        hs = range(part[1], part[1] + 1)
                        part[1] += 1
                        if part[1] == HPC:
                            part[0] = None
                    else:
                        item = pend.pop(0)
                        i, c0, exg, st, sp, jdone = item
                        if st:
                            part[0] = item
                            part[1] = 1
                            hs = range(0, 1)
                        else:
                            hs = range(HPC)
                    for h in hs:
                        nc.tensor.matmul(
                            pv[:, h, c0:CH], vx[:, i, :],
                            exg[h // 2][:, h % 2, c0:CH],
                            start=st, stop=sp)
                    if jdone is not None and part[0] is None:
                        emit_norm(jdone)

                for j in range(NCH):
                    idxs = list(range(4 * j, 4 * j + 4)) + list(range(4 * j))
                    nlast = len(idxs) - 1
                    for idx, i in enumerate(idxs):
                        r = i - 4 * j if i >= 4 * j else None
                        c0 = PT * r if r is not None else 0
                        ktsl = kt[:, i * PT:(i + 1) * PT]
                        exg = []
                        for g in range(2):
                            diag = r is not None
                            for hh in range(2):
                                h = 2 * g + hh
                                nc.tensor.matmul(
                                    scg[g][:, hh, c0:CH], ktsl,
                                    qtc[j][:, h * CH + c0:(h + 1) * CH],
                                    start=True, stop=not diag)
                            if diag:
                                # additive causal mask: sc[s, c'] += -1e4
                                # for c' < s on the triangle block, so exp
                                # yields exact zeros (nothing extra in the
                                # exp -> pv chain)
                                for hh in range(2):
                                    nc.tensor.matmul(
                                        scg[g][:, hh, c0:c0 + PT], lmask[:],
                                        id128[:], start=False, stop=True)
                            ex = exq.tile([PT, 2, CH], bf16, tag=f"ex{g}",
                                          name=f"ex{g}")
                            nc.scalar.activation(ex[:, :, c0:CH],
                                                 scg[g][:, :, c0:CH],
                                                 Exp, scale=0.125)
                            exg.append(ex)
                        pend.append((i, c0, exg, idx == 0, idx == nlast,
                                     j if idx == nlast else None))
                        # constant lag of 3: a chunk's last pv flushes (and
                        # its norm) interleave with the NEXT chunk's first
                        # score matmuls, so ACT never starves at boundaries.
                        while len(pend) > 3:
                            flush_pv()
                while pend or part[0] is not None:
                    flush_pv()

            # ---- phase D: output projection, dd-interleaved banks ----
            with tc.tile_pool(name="wp", bufs=4, space="PSUM") as wpp, \
                 tc.tile_pool(name="po", bufs=3) as pop:
                n = 0
                for tt in range(KT):
                    wps = [wpp.tile([PT, CH], fp32, tag="wp", name=f"wp{tt}_{dd}")
                           for dd in range(NCH)]
                    for s in range(2):
                        for dd in range(NCH):
                            nc.tensor.matmul(wps[dd][:],
                                             ot[s][:, tt * PT:(tt + 1) * PT],
                                             wo_sb[:, s, dd * CH:(dd + 1) * CH],
                                             start=(s == 0), stop=(s == 1))
                    pout4 = pop.tile([PT, NCH, CH], bf16, tag="po")
                    for dd in range(NCH):
                        if n % 2 == 0:
                            nc.scalar.copy(pout4[:, dd, :], wps[dd][:])
                        else:
                            nc.vector.tensor_copy(pout4[:, dd, :], wps[dd][:])
                        n += 1
                    nc.sync.dma_start(
                        out_d.ap()[tt * PT:(tt + 1) * PT, :], pout4[:])

    nc.compile()
    _cache["nc"] = nc
    return nc


def _host_prep(x, freqs, wq, wk, wv, wo):
    x2d = np.asarray(x, np.float32)[0]                    # [T, D]
    xt = np.ascontiguousarray(x2d.T).astype(BF16)         # [D, T]
    cos = np.cos(np.asarray(freqs, np.float32))           # [T, 32]
    sin = np.sin(np.asarray(freqs, np.float32))
    cs4 = np.ascontiguousarray(np.tile(cos.T, (4, 1)))    # [128, T]
    sn4 = np.ascontiguousarray(np.tile(sin.T, (4, 1)))

    ev, od = np.arange(0, HD, 2), np.arange(1, HD, 2)

    # permE/permO [128, 256]: head h (cols 64h..64h+63): local row r<32 comes
    # from rE row 32h+r, r>=32 from rO row 32h+(r-32)
    permE = np.zeros((PT, 2 * PT), np.float32)
    permO = np.zeros((PT, 2 * PT), np.float32)
    for h in range(HPC):
        for r in range(32):
            permE[32 * h + r, 64 * h + r] = 1.0
            permO[32 * h + r, 64 * h + 32 + r] = 1.0

    ident = np.eye(64, dtype=np.float32)

    # lmask [128, 128]: lmask[k, s] = -1e4 where k < s; via identity rhs
    # the matmul adds -1e4 to score[s, c'] for c' < s (causal mask)
    kk = np.arange(PT)[:, None]
    ss = np.arange(PT)[None, :]
    lmask = np.where(kk < ss, -1.0e4, 0.0).astype(np.float32)
    id128 = np.eye(PT, dtype=np.float32)

    wq_f = np.asarray(wq, np.float32)
    wk_f = np.asarray(wk, np.float32)
    wv_f = np.asarray(wv, np.float32)
    wo_f = np.asarray(wo, np.float32)

    in_maps = []
    for c in range(NCORES):
        # wq for 4 heads, evens-major-across-heads packing:
        # cols 0:128 = [h0 evens, h1 evens, h2 evens, h3 evens], 128:256 odds
        blocks = [wq_f[:, (c * HPC + h) * HD:(c * HPC + h + 1) * HD] for h in range(HPC)]
        wq_c = np.concatenate([b[:, ev] for b in blocks] + [b[:, od] for b in blocks], axis=1)
        kblk = wk_f[:, c * HD:(c + 1) * HD]
        wkv_c = np.concatenate([kblk[:, ev], kblk[:, od],
                                wv_f[:, c * HD:(c + 1) * HD]], axis=1)
        wo_c = wo_f[c * HPC * HD:(c + 1) * HPC * HD, :]
        # pre-tile to [128 partitions, k-major] so device DMAs are contiguous
        wq_c = wq_c.reshape(KT, PT, HPC * HD).transpose(1, 0, 2).reshape(PT, -1)
        wkv_c = wkv_c.reshape(KT, PT, 2 * HD).transpose(1, 0, 2).reshape(PT, -1)
        wo_c = wo_c.reshape(2, PT, D).transpose(1, 0, 2).reshape(PT, -1)
        in_maps.append({
            "xt": xt,
            "wq": np.ascontiguousarray(wq_c).astype(BF16),
            "wkv": np.ascontiguousarray(wkv_c).astype(BF16),
            "wo": np.ascontiguousarray(wo_c).astype(BF16),
            "cs4": cs4.astype(BF16),
            "sn4": sn4.astype(BF16),
            "ident": ident.astype(BF16),
            "lmask": lmask.astype(BF16),
            "id128": id128.astype(BF16),
        })
    return in_maps


def run(inputs, trace=False, tmpdir=None):
    nc = _build_nc()
    in_maps = _host_prep(**inputs)
    res = run_bass_kernel_spmd(nc, in_maps, list(range(NCORES)),
                               trace=trace, tmpdir=tmpdir)
    acc = np.zeros((T, D), np.float32)
    for c in range(NCORES):
        acc += res.results[c]["partial"].astype(np.float32)
    return acc[None], res


def kernel(**inputs):
    out, _ = run(inputs, trace=False)
    return out


# revision 30
# speedup vs baseline: 1.1146x; 1.0279x over previous
"""GQA attention (B=1, T=2048, D=2048, H=32, KVH=8, HD=64) on 8 TRN2 cores.

Head-tensor-parallel: core c owns kv-head c and q-heads 4c..4c+3.
wq/wk/wv column-parallel, wo row-parallel; partials summed on host.

Pipeline: A (kv proj + rope-k + v transpose) and B (q proj + rope-q +
perm) interleaved per 512-column chunk with KV/E/O accumulation
round-robined across PSUM banks; C (attention) with two head-pair
streams so ACT exp overlaps PE, causal diagonal blocks column-restricted;
vx carries 64 ones-columns so the pv matmul broadcasts the softmax
denominator for free; D (wo proj) with dd-interleaved PSUM banks.
"""
import sys

if "/opt/trn_rl_repo" not in sys.path:
    sys.path.insert(0, "/opt/trn_rl_repo")

import numpy as np
import ml_dtypes

import concourse.bacc as bacc
import concourse.mybir as mybir
import concourse.tile as tile
from concourse.bass_utils import run_bass_kernel_spmd

BF16 = ml_dtypes.bfloat16
T, D, H, KVH, HD = 2048, 2048, 32, 8, 64
NCORES = 8
HPC = H // NCORES            # 4 q heads per core
KT, PT = 16, 128             # k-tiles of 128 over D
NCH = 4                      # t chunks of 512
CH = 512

_cache = {}


def _build_nc():
    if "nc" in _cache:
        return _cache["nc"]
    fp32, bf16 = mybir.dt.float32, mybir.dt.bfloat16
    Exp = mybir.ActivationFunctionType.Exp
    mult = mybir.AluOpType.mult
    nc = bacc.Bacc("TRN2", target_bir_lowering=False, debug=False,
                   num_devices=NCORES)

    xt_d = nc.dram_tensor("xt", [D, T], bf16, kind="ExternalInput")
    wq_d = nc.dram_tensor("wq", [PT, KT * HPC * HD], bf16, kind="ExternalInput")
    wkv_d = nc.dram_tensor("wkv", [PT, KT * 2 * HD], bf16, kind="ExternalInput")
    wo_d = nc.dram_tensor("wo", [PT, 2 * D], bf16, kind="ExternalInput")
    cs4_d = nc.dram_tensor("cs4", [PT, T], bf16, kind="ExternalInput")
    sn4_d = nc.dram_tensor("sn4", [PT, T], bf16, kind="ExternalInput")
    id_d = nc.dram_tensor("ident", [64, 64], bf16, kind="ExternalInput")
    lm_d = nc.dram_tensor("lmask", [PT, PT], bf16, kind="ExternalInput")
    i128_d = nc.dram_tensor("id128", [PT, PT], bf16, kind="ExternalInput")
    out_d = nc.dram_tensor("partial", [T, D], bf16, kind="ExternalOutput")

    with tile.TileContext(nc) as tc:
        with tc.tile_pool(name="const", bufs=1) as const, \
             tc.tile_pool(name="xtp", bufs=4 * KT) as xtp, \
             tc.tile_pool(name="persist", bufs=1) as persist:

            # ---- DMA loads; sync queue: weights/consts + chunk-0 x,
            # gpsimd queue: gpsimd lib warmup then x chunks 1-3 (queues
            # generate descriptors serially, so split across engines) ----
            wkv_sb = const.tile([PT, KT, 2 * HD], bf16, tag="wkv")
            nc.sync.dma_start(wkv_sb[:], wkv_d.ap())
            wq_sb = const.tile([PT, KT, HPC * HD], bf16, tag="wq")
            # x tiles split across the sync and gpsimd queues (each queue
            # issues descriptors serially at ~0.7us) in chunk-priority
            # order so chunk j lands before chunk j+1 starts competing.
            xt = [[None] * KT for _ in range(NCH)]
            consts_emitted = [False]

            def _emit_consts():
                consts_emitted[0] = True
                out = {}
                out["cs4"] = const.tile([PT, T], bf16, tag="cs4", name="cs4")
                nc.sync.dma_start(out["cs4"][:], cs4_d.ap())
                out["sn4"] = const.tile([PT, T], bf16, tag="sn4", name="sn4")
                nc.sync.dma_start(out["sn4"][:], sn4_d.ap())
                out["ident"] = const.tile([64, 64], bf16, tag="ident", name="ident")
                nc.sync.dma_start(out["ident"][:], id_d.ap())
                out["lmask"] = const.tile([PT, PT], bf16, tag="lmask", name="lmask")
                nc.sync.dma_start(out["lmask"][:], lm_d.ap())
                out["id128"] = const.tile([PT, PT], bf16, tag="id128", name="id128")
                nc.sync.dma_start(out["id128"][:], i128_d.ap())
                return out

            for j in range(NCH):
                for k in range(KT):
                    t_ = xtp.tile([PT, CH], bf16, tag="xt", name=f"xt_{j}_{k}")
                    eng = nc.sync if k % 2 == 0 else nc.gpsimd
                    eng.dma_start(
                        t_[:], xt_d.ap()[k * PT:(k + 1) * PT, j * CH:(j + 1) * CH])
                    xt[j][k] = t_
                    if j == 0 and k == 7:
                        # wq lands after the first chunk-0 x tiles: the E
                        # matmuls only need it once KV(0) is underway
                        nc.sync.dma_start(wq_sb[:], wq_d.ap())
                if j == 0:
                    cm = _emit_consts()
            cs4, sn4, ident = cm["cs4"], cm["sn4"], cm["ident"]
            lmask, id128 = cm["lmask"], cm["id128"]
            wo_sb = const.tile([PT, 2, D], bf16, tag="wo")
            nc.sync.dma_start(wo_sb[:], wo_d.ap())

            # ---- persistent SBUF activations ----
            kt = persist.tile([64, T], bf16, tag="kt")
            # vx: [seq, v-dims 0:64 | ones 64:128]; the ones columns make the
            # pv matmul emit the softmax denominator broadcast to rows 64:128.
            vx = persist.tile([PT, KT, PT], bf16, tag="vx")
            nc.vector.memset(vx[:, :, HD:PT], 1.0)
            qtc = [persist.tile([64, HPC * CH], bf16, tag=f"qtc{j}", name=f"qtc{j}")
                   for j in range(NCH)]
            ot = [persist.tile([PT, T], bf16, tag=f"ot{s}", name=f"ot{s}")
                  for s in range(2)]

            # ---- phase A || B: projections + rope, chunk by chunk ----
            # PSUM banks: kv bufs=2 (2) + E,O bufs=2 (4) + vtr (1) = 7
            with tc.tile_pool(name="kvp", bufs=2, space="PSUM") as kvp, \
                 tc.tile_pool(name="eop", bufs=2, space="PSUM") as eop, \
                 tc.tile_pool(name="vtp", bufs=1, space="PSUM") as vtp, \
                 tc.tile_pool(name="tmpab", bufs=2) as tmpab:
                vts = {}

                def emit_tr(jj):
                    # v transpose for chunk jj (vt ready by now)
                    vt = vts.pop(jj)
                    vtr = vtp.tile([PT, 4, HD], bf16, tag="vtr")
                    for u in range(4):
                        nc.tensor.transpose(vtr[:, u, :], vt[:, u * PT:(u + 1) * PT],
                                            ident[:])
                    nc.scalar.copy(vx[:, 4 * jj:4 * jj + 4, 0:HD], vtr[:])

                for j in range(NCH):
                    jsl = slice(j * CH, (j + 1) * CH)
                    KV = kvp.tile([PT, CH], fp32, tag="kv", name=f"kv{j}")
                    E = eop.tile([PT, CH], fp32, tag="E", name=f"E{j}")
                    O = eop.tile([PT, CH], fp32, tag="O", name=f"O{j}")
                    if j >= 1:
                        emit_tr(j - 1)
                    # KV/E/O k-loop round-robined across 3 banks (hides the
                    # psum read-modify-write latency of accumulation). Chunk 0
                    # is DMA-paced: run KV alone first so the E matmuls never
                    # block the in-order PE queue waiting for wq to land.
                    if j == 0:
                        for k in range(KT):
                            nc.tensor.matmul(KV[:], wkv_sb[:, k, :], xt[j][k][:],
                                             start=(k == 0), stop=(k == KT - 1))
                        for k in range(KT):
                            st, sp = (k == 0), (k == KT - 1)
                            nc.tensor.matmul(E[:], wq_sb[:, k, 0:PT], xt[j][k][:],
                                             start=st, stop=sp)
                            nc.tensor.matmul(O[:], wq_sb[:, k, PT:2 * PT],
                                             xt[j][k][:], start=st, stop=sp)
                    else:
                        for k in range(KT):
                            st, sp = (k == 0), (k == KT - 1)
                            nc.tensor.matmul(KV[:], wkv_sb[:, k, :], xt[j][k][:],
                                             start=st, stop=sp)
                            nc.tensor.matmul(E[:], wq_sb[:, k, 0:PT], xt[j][k][:],
                                             start=st, stop=sp)
                            nc.tensor.matmul(O[:], wq_sb[:, k, PT:2 * PT],
                                             xt[j][k][:], start=st, stop=sp)
                    # v extract: vt copy on ACT
                    vt = tmpab.tile([64, CH], bf16, tag="vt")
                    nc.scalar.copy(vt[:], KV[64:PT, :])
                    vts[j] = vt
                    # rope-q (DVE) first; the last chunk's temporaries live
                    # in the persist pool (their reads are deferred into C)
                    tp = tmpab if j < NCH - 1 else persist
                    t1 = tp.tile([PT, CH], fp32, tag="t1", name=f"t1_{j}")
                    t3 = tp.tile([PT, CH], fp32, tag="t3", name=f"t3_{j}")
                    nc.vector.tensor_tensor(t1[:], E[:], cs4[:, jsl], mult)
                    nc.vector.tensor_tensor(t3[:], E[:], sn4[:, jsl], mult)
                    t2 = tp.tile([PT, CH], fp32, tag="t2", name=f"t2_{j}")
                    t4 = tp.tile([PT, CH], fp32, tag="t4", name=f"t4_{j}")
                    nc.vector.tensor_tensor(t2[:], O[:], sn4[:, jsl], mult)
                    nc.vector.tensor_tensor(t4[:], O[:], cs4[:, jsl], mult)
                    # rope-k mults next: last readers of the KV psum bank
                    # (release it early); the kt / qtc writes are pure-SBUF
                    # and only needed by phase C, so they trail
                    k1 = tp.tile([32, CH], fp32, tag="k1", name=f"k1_{j}")
                    k2 = tp.tile([32, CH], fp32, tag="k2", name=f"k2_{j}")
                    k3 = tp.tile([32, CH], fp32, tag="k3", name=f"k3_{j}")
                    k4 = tp.tile([32, CH], fp32, tag="k4", name=f"k4_{j}")
                    nc.vector.tensor_tensor(k1[:], KV[0:32, :], cs4[0:32, jsl], mult)
                    nc.vector.tensor_tensor(k2[:], KV[32:64, :], sn4[0:32, jsl], mult)
                    nc.vector.tensor_tensor(k3[:], KV[0:32, :], sn4[0:32, jsl], mult)
                    nc.vector.tensor_tensor(k4[:], KV[32:64, :], cs4[0:32, jsl], mult)
                    def kt_qtc_writes(jj=j, jsl=jsl, k1=k1, k2=k2, k3=k3,
                                      k4=k4, t1=t1, t2=t2, t3=t3, t4=t4):
                        nc.vector.tensor_sub(kt[0:32, jsl], k1[:], k2[:])
                        nc.vector.tensor_add(kt[32:64, jsl], k3[:], k4[:])
                        # write qtc directly: head h rows 0:32 = t1-t2 rows
                        # 32h.., rows 32:64 = t3+t4 (inputs share a start
                        # partition; output start may differ)
                        for h in range(HPC):
                            hp = slice(32 * h, 32 * h + 32)
                            nc.vector.tensor_sub(
                                qtc[jj][0:32, h * CH:(h + 1) * CH],
                                t1[hp, :], t2[hp, :])
                            nc.vector.tensor_add(
                                qtc[jj][32:64, h * CH:(h + 1) * CH],
                                t3[hp, :], t4[hp, :])
                    if j < NCH - 1:
                        kt_qtc_writes()
                    else:
                        # last chunk: defer the pure-SBUF kt/qtc writes into
                        # phase C (DVE is idle there) so the DVE tail only
                        # holds the psum-bank-releasing reads
                        deferred_ktq = kt_qtc_writes
                emit_tr(NCH - 1)

            # ---- phase C: attention, software-pipelined ----
            # PSUM: scA/scB [128,2,512] = 2+2 banks, pv [128,4,512] = 4 banks.
            # Two head-pair streams so exp(pairA) overlaps scores(pairB); pv
            # matmuls lag one i behind scores. Per chunk j: diagonal blocks
            # first (col-restricted to the causal-live columns).
            with tc.tile_pool(name="scp", bufs=2, space="PSUM") as scp, \
                 tc.tile_pool(name="pvp", bufs=1, space="PSUM") as pvp, \
                 tc.tile_pool(name="exq", bufs=4) as exq, \
                 tc.tile_pool(name="nrm", bufs=2) as nrm:
                scg = [scp.tile([PT, 2, CH], fp32, tag="sc", name=f"sc{g}")
                       for g in range(2)]
                pv = pvp.tile([PT, HPC, CH], fp32, tag="pv")
                deferred_ktq()
                pend = []  # (i, c0, [exA, exB], start, stop, j_done)

                def emit_norm(jj):
                    # normalization per head: denominator rows -> SBUF ->
                    # recip (reciprocal_approx_fast requires SBUF input),
                    # values scaled straight from psum.
                    for h in range(HPC):
                        dn = nrm.tile([64, CH], fp32, tag="dn")
                        nc.vector.tensor_copy(dn[:], pv[64:PT, h, :])
                        rcp = nrm.tile([64, CH], fp32, tag="rcp")
                        nc.vector.reciprocal_approx_fast(rcp[:], dn[:])
                        nc.vector.tensor_tensor(
                            ot[h // 2][64 * (h % 2):64 * (h % 2) + 64,
                                       jj * CH:(jj + 1) * CH],
                            pv[0:HD, h, :], rcp[:], mult)

                part = [None, 0]

                def flush_pv():
                    # chunk-start items flush one head per call so the
                    # fresh norm gets time to release each pv bank
                    if part[0] is not None:
                        i, c0, exg, st, sp, jdone = part[0]
                        hs = range(part[1], part[1] + 1)
                        part[1] += 1
                        if part[1] == HPC:
                            part[0] = None
                    else:
                        item = pend.pop(0)
                        i, c0, exg, st, sp, jdone = item
                        if st:
                            part[0] = item
                            part[1] = 1
                            hs = range(0, 1)
                        else:
                            hs = range(HPC)
                    for h in hs:
                        nc.tensor.matmul(
                            pv[:, h, c0:CH], vx[:, i, :],
                            exg[h // 2][:, h % 2, c0:CH],
                            start=st, stop=sp)
                    if jdone is not None and part[0] is None:
                        emit_norm(jdone)

                for j in range(NCH):
                    idxs = list(range(4 * j, 4 * j + 4)) + list(range(4 * j))
                    nlast = len(idxs) - 1
                    for idx, i in enumerate(idxs):
                        r = i - 4 * j if i >= 4 * j else None
                        c0 = PT * r if r is not None else 0
                        ktsl = kt[:, i * PT:(i + 1) * PT]
                        exg = []
                        for g in range(2):
                            diag = r is not None
                            for hh in range(2):
                                h = 2 * g + hh
                                nc.tensor.matmul(
                                    scg[g][:, hh, c0:CH], ktsl,
                                    qtc[j][:, h * CH + c0:(h + 1) * CH],
                                    start=True, stop=not diag)
                            if diag:
                                # additive causal mask: sc[s, c'] += -1e4
                                # for c' < s on the triangle block, so exp
                                # yields exact zeros (nothing extra in the
                                # exp -> pv chain)
                                for hh in range(2):
                                    nc.tensor.matmul(
                                        scg[g][:, hh, c0:c0 + PT], lmask[:],
                                        id128[:], start=False, stop=True)
                            ex = exq.tile([PT, 2, CH], bf16, tag=f"ex{g}",
                                          name=f"ex{g}")
                            nc.scalar.activation(ex[:, :, c0:CH],
                                                 scg[g][:, :, c0:CH],
                                                 Exp, scale=0.125)
                            exg.append(ex)
                        pend.append((i, c0, exg, idx == 0, idx == nlast,
                                     j if idx == nlast else None))
                        # constant lag of 3: a chunk's last pv flushes (and
                        # its norm) interleave with the NEXT chunk's first
                        # score matmuls, so ACT never starves at boundaries.
                        while len(pend) > 3:
                            flush_pv()
                while pend or part[0] is not None:
                    flush_pv()

            # ---- phase D: output projection, dd-interleaved banks ----
            with tc.tile_pool(name="wp", bufs=4, space="PSUM") as wpp, \
                 tc.tile_pool(name="po", bufs=3) as pop:
                n = 0
                for tt in range(KT):
                    wps = [wpp.tile([PT, CH], fp32, tag="wp", name=f"wp{tt}_{dd}")
                           for dd in range(NCH)]
                    for s in range(2):
                        for dd in range(NCH):
                            nc.tensor.matmul(wps[dd][:],
                                             ot[s][:, tt * PT:(tt + 1) * PT],
                                             wo_sb[:, s, dd * CH:(dd + 1) * CH],
                                             start=(s == 0), stop=(s == 1))
                    pout4 = pop.tile([PT, NCH, CH], bf16, tag="po")
                    for dd in range(NCH):
                        if n % 2 == 0:
                            nc.scalar.copy(pout4[:, dd, :], wps[dd][:])
                        else:
                            nc.vector.tensor_copy(pout4[:, dd, :], wps[dd][:])
                        n += 1
                    nc.sync.dma_start(
                        out_d.ap()[tt * PT:(tt + 1) * PT, :], pout4[:])

    nc.compile()
    _cache["nc"] = nc
    return nc


def _host_prep(x, freqs, wq, wk, wv, wo):
    x2d = np.asarray(x, np.float32)[0]                    # [T, D]
    xt = np.ascontiguousarray(x2d.T).astype(BF16)         # [D, T]
    cos = np.cos(np.asarray(freqs, np.float32))           # [T, 32]
    sin = np.sin(np.asarray(freqs, np.float32))
    cs4 = np.ascontiguousarray(np.tile(cos.T, (4, 1)))    # [128, T]
    sn4 = np.ascontiguousarray(np.tile(sin.T, (4, 1)))

    ev, od = np.arange(0, HD, 2), np.arange(1, HD, 2)

    # permE/permO [128, 256]: head h (cols 64h..64h+63): local row r<32 comes
    # from rE row 32h+r, r>=32 from rO row 32h+(r-32)
    permE = np.zeros((PT, 2 * PT), np.float32)
    permO = np.zeros((PT, 2 * PT), np.float32)
    for h in range(HPC):
        for r in range(32):
            permE[32 * h + r, 64 * h + r] = 1.0
            permO[32 * h + r, 64 * h + 32 + r] = 1.0

    ident = np.eye(64, dtype=np.float32)

    # lmask [128, 128]: lmask[k, s] = -1e4 where k < s; via identity rhs
    # the matmul adds -1e4 to score[s, c'] for c' < s (causal mask)
    kk = np.arange(PT)[:, None]
    ss = np.arange(PT)[None, :]
    lmask = np.where(kk < ss, -1.0e4, 0.0).astype(np.float32)
    id128 = np.eye(PT, dtype=np.float32)

    wq_f = np.asarray(wq, np.float32)
    wk_f = np.asarray(wk, np.float32)
    wv_f = np.asarray(wv, np.float32)
    wo_f = np.asarray(wo, np.float32)

    in_maps = []
    for c in range(NCORES):
        # wq for 4 heads, evens-major-across-heads packing:
        # cols 0:128 = [h0 evens, h1 evens, h2 evens, h3 evens], 128:256 odds
        blocks = [wq_f[:, (c * HPC + h) * HD:(c * HPC + h + 1) * HD] for h in range(HPC)]
        wq_c = np.concatenate([b[:, ev] for b in blocks] + [b[:, od] for b in blocks], axis=1)
        kblk = wk_f[:, c * HD:(c + 1) * HD]
        wkv_c = np.concatenate([kblk[:, ev], kblk[:, od],
                                wv_f[:, c * HD:(c + 1) * HD]], axis=1)
        wo_c = wo_f[c * HPC * HD:(c + 1) * HPC * HD, :]
        # pre-tile to [128 partitions, k-major] so device DMAs are contiguous
        wq_c = wq_c.reshape(KT, PT, HPC * HD).transpose(1, 0, 2).reshape(PT, -1)
        wkv_c = wkv_c.reshape(KT, PT, 2 * HD).transpose(1, 0, 2).reshape(PT, -1)
        wo_c = wo_c.reshape(2, PT, D).transpose(1, 0, 2).reshape(PT, -1)
        in_maps.append({
            "xt": xt,
            "wq": np.ascontiguousarray(wq_c).astype(BF16),
            "wkv": np.ascontiguousarray(wkv_c).astype(BF16),
            "wo": np.ascontiguousarray(wo_c).astype(BF16),
            "cs4": cs4.astype(BF16),
            "sn4": sn4.astype(BF16),
            "ident": ident.astype(BF16),
            "lmask": lmask.astype(BF16),
            "id128": id128.astype(BF16),
        })
    return in_maps


def run(inputs, trace=False, tmpdir=None):
    nc = _build_nc()
    in_maps = _host_prep(**inputs)
    res = run_bass_kernel_spmd(nc, in_maps, list(range(NCORES)),
                               trace=trace, tmpdir=tmpdir)
    acc = np.zeros((T, D), np.float32)
    for c in range(NCORES):
        acc += res.results[c]["partial"].astype(np.float32)
    return acc[None], res


def kernel(**inputs):
    out, _ = run(inputs, trace=False)
    return out
